# revision 1
# baseline (speedup 1.0000x reference)
"""Trainium2 Bass kernel for sliding-window (±64) multi-head attention.

Reference computation (seq=4096, hidden=768, 12 heads x 64, RoPE, window 128):
    qkv = qkv_weight @ x ; q,k = rope(q,k) ; scores = q^T k / 8 + band_mask
    attn = softmax(scores) @ v ; out = out_weight @ attn

Sharding: sequence-parallel over 8 cores. Core c owns queries
[512c, 512c+512) and computes K/V over the haloed span [512c-64, 512c+576)
(zero-padded at the sequence edges; padding is killed by the band mask).
Each core runs an identical Bass program on different data; the full output
is reassembled on host by concatenation (no collectives needed).

Engine notes: DVE/ACT lanes are partition-fixed, so rotate_half (a +-32
partition swap) is done as a PE matmul against a signed permutation matrix;
P^T is produced by a PE matmul against the identity, with the softmax
normalization applied beforehand as a per-partition tensor_scalar multiply.
Attention works on whole head pairs ([128, 512] tiles) to amortize the
per-op access latency of DVE/ACT.
"""

import os
import sys

import numpy as np

for _p in ("/opt/trn_rl_repo",):
    if _p not in sys.path and os.path.isdir(_p):
        sys.path.insert(0, _p)

import ml_dtypes

import concourse.bass as bass
import concourse.bacc as bacc
import concourse.tile as tile
from concourse import mybir
from concourse.bass_utils import run_bass_kernel_spmd

F32 = mybir.dt.float32
F32R = mybir.dt.float32r
BF16 = mybir.dt.bfloat16

N_CORES = 8
SEQ = 4096
S_CORE = SEQ // N_CORES  # 512 queries per core
HALO = 64                # window // 2
SPAN = S_CORE + 2 * HALO  # 640 keys per core
HID = 768
NH = 12
DH = 64
NCH = HID // 128         # 6 contraction chunks
NHP = NH // 2            # 6 head pairs
NQB = S_CORE // 128      # 4 query blocks per core
NSC = SPAN // 128        # 5 key chunks per core
KSPAN = 256              # key span per query block

_BUILD_CACHE = {}


def _build(add_mask: bool, reps: int = 1):
    """Build + compile the per-core Bass program (shared by all 8 cores).

    reps>1 unrolls the whole kernel body (incl. input DMA) that many times
    inside one program — used only by the timing harness.
    """
    nc = bacc.Bacc("TRN2", target_bir_lowering=False, debug=False, num_devices=N_CORES)

    xin = nc.dram_tensor("xin", [128, NCH * SPAN], BF16, kind="ExternalInput")
    wqt = nc.dram_tensor("wqt", [128, NCH * HID], BF16, kind="ExternalInput")
    wkt = nc.dram_tensor("wkt", [128, NCH * HID], BF16, kind="ExternalInput")
    wvt = nc.dram_tensor("wvt", [128, NCH * HID], BF16, kind="ExternalInput")
    wot = nc.dram_tensor("wot", [128, NCH * HID], BF16, kind="ExternalInput")
    cosb = nc.dram_tensor("cosb", [128, SPAN], F32, kind="ExternalInput")
    sinp = nc.dram_tensor("sinp", [128, SPAN], F32, kind="ExternalInput")
    perms = nc.dram_tensor("perms", [128, 128], F32R, kind="ExternalInput")
    maskb = nc.dram_tensor("maskb", [128, NQB * 2 * KSPAN], BF16, kind="ExternalInput")
    if add_mask:
        maskf = nc.dram_tensor(
            "maskf", [128, NQB * 2 * KSPAN], F32, kind="ExternalInput"
        )
    diag = nc.dram_tensor("diag", [128, 128], BF16, kind="ExternalInput")
    out_d = nc.dram_tensor("out", [128, NCH * S_CORE], F32, kind="ExternalOutput")

    mult = mybir.AluOpType.mult
    addop = mybir.AluOpType.add
    exp = mybir.ActivationFunctionType.Exp

    with tile.TileContext(nc) as tc:
        from contextlib import ExitStack

        for _rep in range(reps):
          with ExitStack() as ctx:
            const = ctx.enter_context(tc.tile_pool(name="const", bufs=1))
            sb = ctx.enter_context(tc.tile_pool(name="sb", bufs=1))
            tmp = ctx.enter_context(tc.tile_pool(name="tmp", bufs=4))
            attnp = ctx.enter_context(tc.tile_pool(name="attnp", bufs=6))
            scal = ctx.enter_context(tc.tile_pool(name="scal", bufs=6))
            outp = ctx.enter_context(tc.tile_pool(name="outp", bufs=2))
            ps_proj = ctx.enter_context(
                tc.tile_pool(name="ps_proj", bufs=2, space="PSUM")
            )
            ps_att = ctx.enter_context(
                tc.tile_pool(name="ps_att", bufs=5, space="PSUM")
            )
            ps_o = ctx.enter_context(tc.tile_pool(name="ps_o", bufs=1, space="PSUM"))

            # ---- input DMAs, ordered by first use ----
            # X and WVT per-chunk (VT projection runs first); the rest whole.
            Xc = []
            WVTc = []
            for k in range(NCH):
                xk = const.tile([128, SPAN], BF16, tag=f"X{k}")
                nc.sync.dma_start(out=xk[:], in_=xin.ap()[:, k * SPAN : (k + 1) * SPAN])
                Xc.append(xk)
                wk_ = const.tile([128, HID], BF16, tag=f"WVT{k}")
                nc.sync.dma_start(
                    out=wk_[:], in_=wvt.ap()[:, k * HID : (k + 1) * HID]
                )
                WVTc.append(wk_)
            def load_whp(src_ap, hp, tagpfx):
                t = const.tile([128, NCH * 128], BF16, tag=f"{tagpfx}{hp}")
                nc.sync.dma_start(
                    out=t[:],
                    in_=src_ap[:, hp * NCH * 128 : (hp + 1) * NCH * 128],
                )
                return t

            COS = const.tile([128, SPAN], F32, tag="COS")
            nc.sync.dma_start(out=COS[:], in_=cosb.ap())
            SINP = const.tile([128, SPAN], F32, tag="SINP")
            nc.sync.dma_start(out=SINP[:], in_=sinp.ap())
            PERMS = const.tile([128, 128], F32R, tag="PERMS")
            nc.sync.dma_start(out=PERMS[:], in_=perms.ap())
            MB = const.tile([128, NQB * 2 * KSPAN], BF16, tag="MB")
            nc.sync.dma_start(out=MB[:], in_=maskb.ap())
            if add_mask:
                MF = const.tile([128, NQB * 2 * KSPAN], F32, tag="MF")
                nc.sync.dma_start(out=MF[:], in_=maskf.ap())
            DIAG = const.tile([128, 128], BF16, tag="DIAG")
            nc.sync.dma_start(out=DIAG[:], in_=diag.ap())
            WQc = {}
            WKc = {}
            for hp_ in range(NHP):
                WQc[hp_] = load_whp(wqt.ap(), hp_, "WQ")
                WKc[hp_] = load_whp(wkt.ap(), hp_, "WK")
            WOT = sb.tile([128, NCH * HID], BF16, tag="WOT")
            nc.sync.dma_start(out=WOT[:], in_=wot.ap())

            # persistent intermediates
            Qs = sb.tile([128, NHP * S_CORE], F32R, tag="Qs")   # [2hd, (hp, s)]
            Ks = sb.tile([128, NHP * SPAN], F32R, tag="Ks")     # [2hd, (hp, s)]
            VT = sb.tile([128, NSC * HID], BF16, tag="VT")      # [s, (chunk, hd)]
            AT = sb.tile([128, NCH * S_CORE], BF16, tag="AT")   # [c, (cchunk, s)]

            # ---- V^T projection: VT[s, hd] per 128-key chunk (bf16) ----
            def vt_unit(sc):
                for hf in range(2):
                    w = HID // 2  # 384
                    vp = ps_proj.tile([128, w], F32, tag="proj")
                    for k in range(NCH):
                        nc.tensor.matmul(
                            vp[:],
                            Xc[k][:, sc * 128 : (sc + 1) * 128],
                            WVTc[k][:, hf * w : (hf + 1) * w],
                            start=(k == 0),
                            stop=(k == NCH - 1),
                        )
                    nc.scalar.copy(
                        VT[:, sc * HID + hf * w : sc * HID + (hf + 1) * w], vp[:]
                    )

            def rope(dst, src_ps, cos_ap, sin_ap, w):
                # dst = src*cos + rot(src)*sin ; rot via PE permutation matmul
                qsb = tmp.tile([128, S_CORE], F32R, tag="ropet")
                nc.scalar.copy(qsb[:, :w], src_ps)
                qrot = ps_proj.tile([128, S_CORE], F32, tag="proj")
                nc.tensor.matmul(
                    qrot[:, :w], PERMS[:], qsb[:, :w], start=True, stop=True
                )
                nc.gpsimd.tensor_tensor(dst, qsb[:, :w], cos_ap, op=mult)
                m2 = tmp.tile([128, S_CORE], F32, tag="ropem")
                nc.vector.tensor_tensor(m2[:, :w], qrot[:, :w], sin_ap, op=mult)
                nc.gpsimd.tensor_tensor(dst, dst, m2[:, :w], op=addop)

            # ---- per head pair: project Q,K then attention, software-
            # pipelined: proj(hp+1) is emitted before attention(hp) so the
            # PE queue always has dependency-free matmuls ahead of the
            # attention ops that wait on DVE/ACT results; likewise scores
            # for qb+1 are emitted before the softmax chain of qb. ----
            def proj_hp(hp):
                qp = ps_proj.tile([128, S_CORE], F32, tag="proj")
                for k in range(NCH):
                    nc.tensor.matmul(
                        qp[:],
                        WQc[hp][:, k * 128 : (k + 1) * 128],
                        Xc[k][:, HALO : HALO + S_CORE],
                        start=(k == 0),
                        stop=(k == NCH - 1),
                    )
                rope(
                    Qs[:, hp * S_CORE : (hp + 1) * S_CORE],
                    qp[:],
                    COS[:, HALO : HALO + S_CORE],
                    SINP[:, HALO : HALO + S_CORE],
                    S_CORE,
                )
                for half in range(2):
                    w = SPAN // 2  # 320
                    kp = ps_proj.tile([128, w], F32, tag="proj")
                    for k in range(NCH):
                        nc.tensor.matmul(
                            kp[:],
                            WKc[hp][:, k * 128 : (k + 1) * 128],
                            Xc[k][:, half * w : (half + 1) * w],
                            start=(k == 0),
                            stop=(k == NCH - 1),
                        )
                    rope(
                        Ks[:, hp * SPAN + half * w : hp * SPAN + (half + 1) * w],
                        kp[:],
                        COS[:, half * w : (half + 1) * w],
                        SINP[:, half * w : (half + 1) * w],
                        w,
                    )

            def attn_scores(hp, qb):
                # one PSUM tile (= one bank) per head: the two matmuls
                # contract over different partition row groups and run
                # concurrently on the PE, so they must drain into
                # different PSUM banks.
                ss = []
                for h in range(2):
                    s1 = ps_att.tile([128, KSPAN], F32, tag="att",
                                     name=f"s_{hp}_{qb}_{h}")
                    nc.tensor.matmul(
                        s1[:],
                        Qs[64 * h : 64 * (h + 1),
                           hp * S_CORE + qb * 128 : hp * S_CORE + (qb + 1) * 128],
                        Ks[64 * h : 64 * (h + 1),
                           hp * SPAN + qb * 128 : hp * SPAN + qb * 128 + KSPAN],
                        start=True,
                        stop=True,
                    )
                    ss.append(s1)
                return ss

            # ---- attention as a 6-stage modulo software pipeline over the
            # 24 (head-pair, query-block) units: at each step, stage k runs
            # for unit i-k, so every engine queue holds ready work and ~6
            # units are in flight. ----
            def stage_exp(st):
                praw2 = attnp.tile([128, 2 * KSPAN], BF16, tag="praw")
                moff = st["qb"] * 2 * KSPAN
                for h in range(2):
                    sh = st["s2"][h]
                    dst = praw2[:, h * KSPAN : (h + 1) * KSPAN]
                    if add_mask:
                        ssb2 = tmp.tile([128, KSPAN], F32, tag="ssb")
                        nc.vector.tensor_tensor(
                            ssb2[:], sh[:],
                            MF[:, moff + h * KSPAN : moff + (h + 1) * KSPAN],
                            op=addop,
                        )
                        nc.scalar.activation(dst, ssb2[:], exp)
                    else:
                        nc.scalar.activation(dst, sh[:], exp)
                st["praw"] = praw2
                del st["s2"]

            def stage_dve(st):
                qb = st["qb"]
                moff = qb * 2 * KSPAN
                praw2 = st["praw"]
                P2 = attnp.tile([128, 2 * KSPAN], BF16, tag="P")
                ssum2 = scal.tile([128, 2], F32, tag="ssum")
                nc.vector.tensor_tensor(
                    P2[:], praw2[:], MB[:, moff : moff + 2 * KSPAN], op=mult
                )
                nc.vector.tensor_reduce(
                    out=ssum2[:],
                    in_=P2[:].rearrange("p (h k) -> p h k", h=2),
                    axis=mybir.AxisListType.X,
                    op=addop,
                )
                rr2 = scal.tile([128, 2], F32, tag="rr")
                nc.vector.reciprocal(rr2[:], ssum2[:])
                P2n = attnp.tile([128, 2 * KSPAN], BF16, tag="Pn")
                for h in range(2):
                    nc.vector.tensor_scalar_mul(
                        P2n[:, h * KSPAN : (h + 1) * KSPAN],
                        P2[:, h * KSPAN : (h + 1) * KSPAN],
                        rr2[:, h : h + 1],
                    )
                st["P2n"] = P2n
                del st["praw"]

            def stage_pt(st):
                P2n = st["P2n"]
                pt2 = ps_att.tile([128, 2 * KSPAN], F32, tag="att")
                for h in range(2):
                    for hf in range(2):
                        off = h * KSPAN + hf * 128
                        nc.tensor.matmul(
                            pt2[:, off : off + 128],
                            P2n[:, off : off + 128],
                            DIAG[:],
                            start=True,
                            stop=True,
                        )
                st["pt2"] = pt2
                del st["P2n"]

            def stage_evac(st):
                pts2 = attnp.tile([128, 2 * KSPAN], BF16, tag="pts")
                nc.scalar.copy(pts2[:], st["pt2"][:])
                st["pts2"] = pts2
                del st["pt2"]

            def stage_pv(st):
                hp, qb = st["hp"], st["qb"]
                if qb == 0:
                    o2s[hp] = ps_o.tile([128, S_CORE], F32, tag="o",
                                        name=f"o2_{hp}")
                o2 = o2s[hp]
                pts2 = st["pts2"]
                for h in range(2):
                    hg = hp * 2 + h
                    osl = o2[64 * h : 64 * (h + 1), qb * 128 : (qb + 1) * 128]
                    tp = (0, 64 * h)
                    nc.tensor.matmul(
                        osl,
                        VT[:, qb * HID + hg * 64 : qb * HID + hg * 64 + 64],
                        pts2[:, h * KSPAN : h * KSPAN + 128],
                        start=True, stop=False, tile_position=tp,
                    )
                    nc.tensor.matmul(
                        osl,
                        VT[:, (qb + 1) * HID + hg * 64 : (qb + 1) * HID + hg * 64 + 64],
                        pts2[:, h * KSPAN + 128 : (h + 1) * KSPAN],
                        start=False, stop=True, tile_position=tp,
                    )
                del st["pts2"]
                if qb == NQB - 1:
                    nc.vector.tensor_copy(
                        AT[:, hp * S_CORE : (hp + 1) * S_CORE], o2[:]
                    )
                    del o2s[hp]

            o2s = {}

            def stage_scores(st):
                st["s2"] = attn_scores(st["hp"], st["qb"])

            PO1 = sb.tile([128, NCH * S_CORE], F32, tag="PO1")

            def outproj_part1():
                for oc in range(NCH):
                    ops = ps_proj.tile([128, S_CORE], F32, tag="proj")
                    for k in range(5):
                        nc.tensor.matmul(
                            ops[:],
                            WOT[:, k * HID + oc * 128 : k * HID + (oc + 1) * 128],
                            AT[:, k * S_CORE : (k + 1) * S_CORE],
                            start=(k == 0),
                            stop=(k == 4),
                        )
                    nc.vector.tensor_copy(
                        PO1[:, oc * S_CORE : (oc + 1) * S_CORE], ops[:]
                    )

            vt_unit(0)
            proj_hp(0)
            vt_unit(1)
            proj_hp(1)
            vt_unit(2)
            vt_unit(3)
            vt_unit(4)
            proj_hp(2)

            units = [
                {"hp": hp, "qb": qb} for hp in range(NHP) for qb in range(NQB)
            ]
            stages = [stage_scores, stage_exp, stage_dve, stage_pt,
                      stage_evac, stage_pv]
            NU = len(units)
            ND = len(stages)
            for step in range(NU + ND - 1):
                # emit remaining projections just before each head pair's
                # first unit enters the pipeline
                if step < NU:
                    hp, qb = units[step]["hp"], units[step]["qb"]
                    if qb == 0 and hp + 3 < NHP and hp + 3 >= 3:
                        proj_hp(hp + 3)
                for k in range(ND - 1, -1, -1):
                    idx = step - k
                    if 0 <= idx < NU:
                        stages[k](units[idx])
                # out-projection chunks 0-4 right after head pair 4 retires
                if step == 5 * 4 - 1 + ND - 1:
                    outproj_part1()

            # ---- output projection (split contraction: chunks 0-3 run as
            # soon as head pairs 0-3 are done; 4-5 + combine at the end) ----
            for oc in range(NCH):
                ops = ps_proj.tile([128, S_CORE], F32, tag="proj")
                for k in range(5, NCH):
                    nc.tensor.matmul(
                        ops[:],
                        WOT[:, k * HID + oc * 128 : k * HID + (oc + 1) * 128],
                        AT[:, k * S_CORE : (k + 1) * S_CORE],
                        start=(k == 5),
                        stop=(k == NCH - 1),
                    )
                ot = outp.tile([128, S_CORE], F32, tag="ot")
                nc.vector.scalar_tensor_tensor(
                    out=ot[:], in0=ops[:], scalar=1.0,
                    in1=PO1[:, oc * S_CORE : (oc + 1) * S_CORE],
                    op0=mult, op1=addop,
                )
                nc.sync.dma_start(
                    out=out_d.ap()[:, oc * S_CORE : (oc + 1) * S_CORE], in_=ot[:]
                )

    nc.compile()
    return nc


def get_program(add_mask: bool, reps: int = 1):
    key = (add_mask, reps)
    if key not in _BUILD_CACHE:
        _BUILD_CACHE[key] = _build(add_mask, reps)
    return _BUILD_CACHE[key]


def _pack_chunked(a, nch, w):
    """[nch*128, w] row-major -> [128, nch*w] with chunk-major free dim."""
    return np.ascontiguousarray(
        a.reshape(nch, 128, w).transpose(1, 0, 2).reshape(128, nch * w)
    )


def prep_core_inputs(core, xs, pos, am, qkv_weight, out_weight, add_mask):
    """Build the per-core input map (numpy) for one core."""
    start = S_CORE * core - HALO
    idx = np.arange(start, start + SPAN)
    valid = (idx >= 0) & (idx < SEQ)

    Xs = np.zeros((HID, SPAN), np.float32)
    Xs[:, valid] = xs[:, idx[valid]]

    pspan = np.zeros((SPAN,), np.float32)
    pspan[valid] = pos[idx[valid]]
    invf = (
        1.0 / (10000.0 ** (np.arange(0, DH, 2, dtype=np.float32) / np.float32(DH)))
    ).astype(np.float32)
    f = pspan[None, :] * invf[:, None]  # [32, SPAN]
    cos32 = np.cos(f).astype(np.float32)
    sin32 = np.sin(f).astype(np.float32)
    COS = np.tile(cos32, (4, 1))
    SINP = np.tile(sin32, (4, 1))

    # signed rotate-half permutation: (PERMS.T @ q)[d] = rot_half(q)[d]
    di = np.arange(128)
    lo = (di % 64) < 32
    src = np.where(lo, di + 32, di - 32)
    sgn = np.where(lo, -1.0, 1.0).astype(np.float32)
    PERMS = np.zeros((128, 128), np.float32)
    PERMS[src, di] = sgn

    # masks, duplicated per head of the pair: [128, (qb, h, 256)]
    mb = np.zeros((128, NQB, 2, KSPAN), np.float32)
    mf = np.full((128, NQB, 2, KSPAN), -10000.0, np.float32)
    for qb in range(NQB):
        qg = S_CORE * core + 128 * qb + np.arange(128)
        kg = S_CORE * core + 128 * qb - HALO + np.arange(KSPAN)
        kvalid = (kg >= 0) & (kg < SEQ)
        band = (np.abs(kg[None, :] - qg[:, None]) <= HALO) & kvalid[None, :]
        mb[:, qb, 0, :] = band
        mb[:, qb, 1, :] = band
        if add_mask:
            amband = np.zeros((128, KSPAN), np.float32)
            amband[:, kvalid] = am[np.ix_(qg, kg[kvalid])]
            m = np.where(band, amband, -10000.0)
            mf[:, qb, 0, :] = m
            mf[:, qb, 1, :] = m

    wq = qkv_weight[0:HID] * np.float32(DH**-0.5)
    wk = qkv_weight[HID : 2 * HID]
    wv = qkv_weight[2 * HID : 3 * HID]

    def packw(w):
        return _pack_chunked(
            np.ascontiguousarray(w.T.astype(ml_dtypes.bfloat16)), NCH, HID
        )

    def packw_hp(w):
        # [c, o] -> [128, (hp, cchunk, 128)] so per-head-pair DMAs are
        # contiguous in the free dimension
        wt = np.ascontiguousarray(w.T.astype(ml_dtypes.bfloat16))  # [768c, 768o]
        a = wt.reshape(NCH, 128, NHP, 128)  # (cchunk, p, hp, n)
        return np.ascontiguousarray(
            a.transpose(1, 2, 0, 3).reshape(128, NHP * NCH * 128)
        )

    in_map = {
        "xin": _pack_chunked(Xs.astype(ml_dtypes.bfloat16), NCH, SPAN),
        "wqt": packw_hp(wq),
        "wkt": packw_hp(wk),
        "wvt": packw(wv),
        "wot": packw(out_weight),
        "cosb": COS,
        "sinp": SINP,
        "perms": PERMS,
        "maskb": mb.reshape(128, NQB * 2 * KSPAN).astype(ml_dtypes.bfloat16),
        "diag": np.eye(128, dtype=ml_dtypes.bfloat16),
    }
    if add_mask:
        in_map["maskf"] = np.ascontiguousarray(mf.reshape(128, NQB * 2 * KSPAN))
    return in_map


def prep_all_inputs(x, position_ids, attention_mask, qkv_weight, out_weight):
    xs = np.asarray(x, dtype=np.float32)[0, :, 0, :]  # [768, 4096]
    pos = np.asarray(position_ids)[0].astype(np.float32)
    am = np.asarray(attention_mask, dtype=np.float32)[0, 0]
    qkv_w = np.asarray(qkv_weight, dtype=np.float32)
    out_w = np.asarray(out_weight, dtype=np.float32)
    add_mask = bool(np.any(am))
    in_maps = [
        prep_core_inputs(c, xs, pos, am, qkv_w, out_w, add_mask)
        for c in range(N_CORES)
    ]
    return in_maps, add_mask


def assemble_output(results):
    cols = []
    for c in range(N_CORES):
        o = np.asarray(results[c]["out"])  # [128, 6*512]
        cols.append(o.reshape(128, NCH, S_CORE).transpose(1, 0, 2).reshape(HID, S_CORE))
    full = np.concatenate(cols, axis=1)  # [768, 4096]
    return np.ascontiguousarray(full.reshape(1, HID, 1, SEQ), dtype=np.float32)


def kernel(**inputs):
    in_maps, add_mask = prep_all_inputs(
        inputs["x"],
        inputs["position_ids"],
        inputs["attention_mask"],
        inputs["qkv_weight"],
        inputs["out_weight"],
    )
    nc = get_program(add_mask)
    res = run_bass_kernel_spmd(nc, in_maps, core_ids=list(range(N_CORES)))
    return assemble_output(res.results)



# revision 49
# speedup vs baseline: 1.2653x; 1.2653x over previous
"""Trainium2 Bass kernel for sliding-window (+-64) multi-head attention. v2.

Reference computation (seq=4096, hidden=768, 12 heads x 64, RoPE, window 128):
    qkv = qkv_weight @ x ; q,k = rope(q,k) ; scores = q^T k / 8 + band_mask
    attn = softmax(scores) @ v ; out = out_weight @ attn

Sharding: sequence-parallel over 8 cores. Core c owns queries
[512c, 512c+512) and computes K/V over the haloed span [512c-64, 512c+576)
(zero-padded at the sequence edges; phantom keys are killed by per-core edge
band-mask tiles). No collectives; host concatenates the 8 outputs.

v2 structure (vs the v1 baseline):
- QKV projections run as fp8(e4m3) DoubleRow matmuls (256-deep contraction,
  0.5 cyc/row) with 3-term error compensation: W ~ Wh+Wl, X ~ Xh+Xl (scaled
  per-tensor by powers of 2), W@X ~ Wh@Xh + Wl@Xh + Wh@Xl.  The 2^-k
  de-scaling folds into the rope cos/sin tables (Q,K) and the V evacuation
  copy scale, so it costs nothing.
- Scores are computed TRANSPOSED ([key, query] in PSUM, stationary = K), so
  softmax's exp output P^T feeds the PV matmul directly: no PE transpose
  and no PSUM evacuation copy.
- The PV stationary is a strided AP [V_h0(64) | ones(64)] (resp.
  [ones | V_h1]): partitions 0-63 of the PV output accumulate attn for h0
  while 64-127 accumulate the softmax denominator (replicated), and
  mirrored for h1.  Normalization is then a small stride-0 DMA replicating
  the denominator row onto the numerator's partitions plus one DVE divide
  per (head, 512 queries) - no reduce, no reciprocal, no transpose.
- rotate_half runs as a 4-piece SBUF->SBUF DMA partition permutation (sign
  folded into the sin table); rope multiplies are bf16 DVE ops at
  [128, 1152] (Q and K fused per head pair).
"""

import os
import sys

import numpy as np

for _p in ("/opt/trn_rl_repo",):
    if _p not in sys.path and os.path.isdir(_p):
        sys.path.insert(0, _p)

import ml_dtypes

import concourse.bass as bass
import concourse.bacc as bacc
import concourse.tile as tile
from concourse import mybir
from concourse.ap import AP
from concourse.bass_utils import run_bass_kernel_spmd

F32 = mybir.dt.float32
F32R = mybir.dt.float32r
BF16 = mybir.dt.bfloat16
FP8 = mybir.dt.float8e4

N_CORES = 8
SEQ = 4096
S_CORE = SEQ // N_CORES  # 512 queries per core
HALO = 64
SPAN = S_CORE + 2 * HALO  # 640 keys per core
HID = 768
NH = 12
DH = 64
NHP = NH // 2            # 6 head pairs
NSC = SPAN // 128        # 5 key chunks per core
NPR = 3                  # fp8 DoubleRow pair-chunks (3 x 256 = 768)
NQB = S_CORE // 128      # 4 query blocks
QKW = S_CORE + SPAN      # 1152: fused q|k rope width per head pair
VTW = NSC * NHP * 192    # VT tile width: per (kc, hp): [V_h0 | ones | V_h1]

DR = mybir.MatmulPerfMode.DoubleRow

_BUILD_CACHE = {}


def _build(add_mask: bool, isv: int, reps: int = 1):
    nc = bacc.Bacc("TRN2", target_bir_lowering=False, debug=False, num_devices=N_CORES)

    xhl_d = nc.dram_tensor("xhl", [128, 2 * NPR * 2 * SPAN], FP8,
                           kind="ExternalInput")
    wqk_d = nc.dram_tensor("wqk", [128, 4 * NHP * NPR * 2 * 128], FP8,
                           kind="ExternalInput")
    wvhl_d = nc.dram_tensor("wvhl", [128, 2 * NPR * 2 * HID], FP8,
                            kind="ExternalInput")
    wot_d = nc.dram_tensor("wot", [128, NHP * 6 * 128], BF16, kind="ExternalInput")
    tabs_d = nc.dram_tensor("tabs", [128, 2 * QKW + 3 * 512], BF16,
                            kind="ExternalInput")
    if add_mask:
        maskf_d = nc.dram_tensor("maskf", [128, NSC * 512], F32, kind="ExternalInput")
    out_d = nc.dram_tensor("out", [128, 6 * S_CORE], BF16, kind="ExternalOutput")

    mult = mybir.AluOpType.mult
    addop = mybir.AluOpType.add
    divop = mybir.AluOpType.divide
    exp = mybir.ActivationFunctionType.Exp
    copyf = mybir.ActivationFunctionType.Copy

    with tile.TileContext(nc) as tc:
        from contextlib import ExitStack

        for _rep in range(reps):
          with ExitStack() as ctx:
            const = ctx.enter_context(tc.tile_pool(name="const", bufs=1))
            sb = ctx.enter_context(tc.tile_pool(name="sb", bufs=1))
            ropep = ctx.enter_context(tc.tile_pool(name="ropep", bufs=2))
            pmp = ctx.enter_context(tc.tile_pool(name="pmp", bufs=6))
            denp = ctx.enter_context(tc.tile_pool(name="denp", bufs=4))
            outp = ctx.enter_context(tc.tile_pool(name="outp", bufs=6))
            ps_proj = ctx.enter_context(
                tc.tile_pool(name="ps_proj", bufs=3, space="PSUM"))
            ps_att = ctx.enter_context(
                tc.tile_pool(name="ps_att", bufs=3, space="PSUM"))
            ps_o = ctx.enter_context(tc.tile_pool(name="ps_o", bufs=2, space="PSUM"))

            # ---- input DMAs (ordered by first use) ----
            XHL = const.tile([128, 2, NPR, 2, SPAN], FP8, tag="XHL")
            nc.sync.dma_start(
                out=XHL[:],
                in_=xhl_d.ap().rearrange(
                    "p (v r t s) -> p v r t s", v=2, r=NPR, t=2))
            XH = XHL[:, 0]
            XL = XHL[:, 1]
            WVHL = const.tile([128, 2, NPR, 2, HID], FP8, tag="WVHL")
            nc.sync.dma_start(
                out=WVHL[:],
                in_=wvhl_d.ap().rearrange(
                    "p (v r t m) -> p v r t m", v=2, r=NPR, t=2))
            WVH = WVHL[:, 0]
            WVL = WVHL[:, 1]
            WQK = const.tile([128, 4, NHP, NPR, 2, 128], FP8, tag="WQK")
            nc.sync.dma_start(
                out=WQK[:],
                in_=wqk_d.ap().rearrange(
                    "p (v k r t m) -> p v k r t m", v=4, k=NHP, r=NPR, t=2))
            WQH, WQL, WKH, WKL = (WQK[:, 0], WQK[:, 1], WQK[:, 2], WQK[:, 3])
            TABS = const.tile([128, 2 * QKW + 3 * 512], BF16, tag="TABS")
            nc.sync.dma_start(out=TABS[:], in_=tabs_d.ap())
            COS = TABS[:, 0:QKW]
            SIN = TABS[:, QKW:2 * QKW]
            BAND = TABS[:, 2 * QKW:].rearrange("p (v h j) -> p v h j", v=3, h=2)
            if add_mask:
                MF = const.tile([128, NSC, 2, 256], F32, tag="MF")
                nc.sync.dma_start(
                    out=MF[:],
                    in_=maskf_d.ap().rearrange("p (k h w) -> p k h w", k=NSC, h=2))

            # persistent intermediates. QKa/QKb hold rope output zero-padded
            # per head (h0 on partitions 0-63 of QKa, h1 on 64-127 of QKb):
            # score matmuls then contract all 128 partitions, so the h0/h1
            # matmuls share one PE row group and may drain into one PSUM
            # bank (concurrent row-split tiles on one bank wedge the HW).
            VT = sb.tile([128, VTW], BF16, tag="VT")
            QKa = sb.tile([128, NHP * QKW], BF16, tag="QKa")
            QKb = sb.tile([128, NHP * QKW], BF16, tag="QKb")
            AT = sb.tile([128, NHP * S_CORE], BF16, tag="AT")
            PO1 = sb.tile([128, 6 * S_CORE], F32, tag="PO1")
            nc.gpsimd.memset(QKa[64:128, :], 0.0)
            nc.gpsimd.memset(QKb[0:64, :], 0.0)

            _vb = VT[:, :]
            nc.gpsimd.memset(
                AP(_vb.tensor, _vb.offset + 64,
                   [list(_vb.ap[0]), [1152, NSC], [192, NHP], [1, 64]]), 1.0)


            # ---- V^T projection (stationary = X pair, moving = Wv pair) ----
            def vt_unit(sc):
                for hf in range(2):
                    vp = ps_proj.tile([128, 384], F32, tag="proj")
                    i = 0
                    for pr in range(NPR):
                        for (wt, xt) in ((WVH, XH), (WVL, XH), (WVH, XL)):
                            nc.tensor.matmul(
                                vp[:],
                                xt[:, pr, :, sc * 128:(sc + 1) * 128],
                                wt[:, pr, :, hf * 384:(hf + 1) * 384],
                                start=(i == 0), stop=(i == NPR * 3 - 1),
                                perf_mode=DR)
                            i += 1
                    # vp cols are host-ordered [h0 of hp(a,b,c) | h1 of same];
                    # scatter into VT's [V_h0 | ones | V_h1] blocks.
                    dst = AP(_vb.tensor,
                             _vb.offset + sc * 1152 + hf * 576,
                             [list(_vb.ap[0]), [128, 2], [192, 3], [1, 64]])
                    nc.scalar.activation(
                        dst,
                        vp[:].rearrange("p (h i j) -> p h i j", h=2, i=3),
                        copyf, scale=float(2.0 ** isv))

            # ---- Q/K projection (stationary = W pair, moving = X pair) ----
            def dr3_w(pap, wgt_h, wgt_l, hp, xbase, w):
                i = 0
                for pr in range(NPR):
                    for (wt, xt) in ((wgt_h, XH), (wgt_l, XH), (wgt_h, XL)):
                        nc.tensor.matmul(
                            pap,
                            wt[:, hp, pr, :, :],
                            xt[:, pr, :, xbase:xbase + w],
                            start=(i == 0), stop=(i == NPR * 3 - 1),
                            perf_mode=DR)
                        i += 1

            def proj_hp(hp):
                qsb = ropep.tile([128, QKW], BF16, tag="qsb")
                qp = ps_proj.tile([128, 512], F32, tag="proj")
                dr3_w(qp[:], WQH, WQL, hp, HALO, 512)
                nc.scalar.copy(qsb[:, 0:512], qp[:])
                for half in range(2):
                    kp = ps_proj.tile([128, 320], F32, tag="proj")
                    dr3_w(kp[:], WKH, WKL, hp, half * 320, 320)
                    nc.scalar.copy(
                        qsb[:, 512 + half * 320:512 + (half + 1) * 320],
                        kp[:])
                # rotate_half as +-32 partition swap (sign folded into SIN)
                qrot = ropep.tile([128, QKW], BF16, tag="qrot")
                for blk in range(2):
                    for half in range(2):
                        src = blk * 64 + (1 - half) * 32
                        dst = blk * 64 + half * 32
                        nc.sync.dma_start(
                            out=qrot[dst:dst + 32, :],
                            in_=qsb[src:src + 32, :])
                t1 = ropep.tile([128, QKW], BF16, tag="t1")
                nc.vector.tensor_tensor(t1[:], qsb[:], COS[:], op=mult)
                m2 = ropep.tile([128, QKW], BF16, tag="m2")
                nc.vector.tensor_tensor(m2[:], qrot[:], SIN[:], op=mult)
                nc.vector.tensor_tensor(
                    QKa[0:64, hp * QKW:(hp + 1) * QKW],
                    t1[0:64, :], m2[0:64, :], op=addop)
                nc.vector.tensor_tensor(
                    QKb[64:128, hp * QKW:(hp + 1) * QKW],
                    t1[64:128, :], m2[64:128, :], op=addop)

            # ---- attention stages over units (hp, kc) ----
            def qwin(kc):
                return max(0, kc * 128 - 128), min(S_CORE, kc * 128 + 128)

            def stage_scores(u):
                hp, kc = u["hp"], u["kc"]
                lo, hi = qwin(kc)
                off = 128 if kc == 0 else 0
                s2 = ps_att.tile([128, 512], F32, tag="att",
                                 name=f"s_{hp}_{kc}")
                for h, QKh in enumerate((QKa, QKb)):
                    nc.tensor.matmul(
                        s2[:, h * 256 + off:h * 256 + off + hi - lo],
                        QKh[:, hp * QKW + 512 + kc * 128:
                            hp * QKW + 512 + (kc + 1) * 128],
                        QKh[:, hp * QKW + lo:hp * QKW + hi],
                        start=True, stop=True)
                u["s2"] = s2

            def stage_exp(u):
                hp, kc = u["hp"], u["kc"]
                lo, hi = qwin(kc)
                off = 128 if kc == 0 else 0
                s2 = u["s2"][:].rearrange("p (h j) -> p h j", h=2)
                pe_ = pmp.tile([128, 2, 256], BF16, tag="pe")
                if add_mask:
                    nc.vector.tensor_tensor(
                        s2[:, :, off:off + hi - lo],
                        s2[:, :, off:off + hi - lo],
                        MF[:, kc, :, off:off + hi - lo],
                        op=addop)
                nc.scalar.activation(
                    pe_[:, :, off:off + hi - lo],
                    s2[:, :, off:off + hi - lo], exp)
                u["pe"] = pe_
                del u["s2"]

            def stage_mask(u):
                hp, kc = u["hp"], u["kc"]
                lo, hi = qwin(kc)
                off = 128 if kc == 0 else 0
                var = 0 if kc == 0 else (2 if kc == NSC - 1 else 1)
                pm_ = pmp.tile([128, 2, 256], BF16, tag="pm",
                               name=f"pm_{hp}_{kc}")
                eng = nc.gpsimd if kc in (0, NSC - 1) else nc.vector
                eng.tensor_tensor(
                    pm_[:, :, off:off + hi - lo],
                    u["pe"][:, :, off:off + hi - lo],
                    BAND[:, var, :, off:off + hi - lo],
                    op=mult)
                u["pm"] = pm_
                del u["pe"]

            def vt_stat(kc, hp, h):
                """PV stationary: h0 -> [V_h0 | ones], h1 -> [ones | V_h1]."""
                pos = kc * 1152 + hp * 192 + h * 64
                return VT[:, pos:pos + 128]

            def stage_pv(u, prev_pm):
                hp, kc = u["hp"], u["kc"]
                qb = kc - 1
                if qb == 0:
                    o2s[hp] = [ps_o.tile([128, 512], F32, tag="o",
                                         name=f"o2_{hp}_{h}") for h in range(2)]
                for h in range(2):
                    o2 = o2s[hp][h]
                    nc.tensor.matmul(
                        o2[:, qb * 128:(qb + 1) * 128],
                        vt_stat(qb, hp, h),
                        prev_pm[:, h, 128:256],
                        start=True, stop=False)
                    nc.tensor.matmul(
                        o2[:, qb * 128:(qb + 1) * 128],
                        vt_stat(qb + 1, hp, h),
                        u["pm"][:, h, 0:128],
                        start=False, stop=True)

            def finish_hp(hp):
                # o2[h0]: attn @ 0-63, den @ 64-127 (replicated)
                # o2[h1]: den @ 0-63 (replicated), attn @ 64-127
                # Engines cannot cross partitions, so hop the replicated den
                # rows to SBUF (ACT copy, aligned) and DMA them onto the
                # numerator's partitions, then divide on DVE.
                o2a, o2b = o2s[hp]
                dsb = denp.tile([128, 512], BF16, tag="dsb")
                with nc.allow_low_precision(reason="softmax recip in bf16"):
                    nc.vector.reciprocal(dsb[64:65, :], o2a[64:65, :])
                    nc.vector.reciprocal(dsb[0:1, :], o2b[0:1, :])
                # replicate 1/den onto the numerator's partitions. HW
                # partition_broadcast needs src AND dst at partition 0, so
                # hop den0 (at partition 64) via a 1-row DMA and broadcast
                # to full-height tiles, reading the needed half.
                trow = denp.tile([1, 512], BF16, tag="trow")
                nc.sync.dma_start(out=trow[0:1, :], in_=dsb[64:65, :])
                rda = denp.tile([128, 512], BF16, tag="rda")
                nc.gpsimd.partition_broadcast(rda[:, :], trow[0:1, :])
                rdb = denp.tile([128, 512], BF16, tag="rdb")
                nc.gpsimd.partition_broadcast(rdb[:, :], dsb[0:1, :])
                nc.vector.tensor_tensor(
                    AT[0:64, hp * S_CORE:(hp + 1) * S_CORE],
                    o2a[0:64, :], rda[0:64, :], op=mult)
                nc.vector.tensor_tensor(
                    AT[64:128, hp * S_CORE:(hp + 1) * S_CORE],
                    o2b[64:128, :], rdb[64:128, :], op=mult)
                del o2s[hp]

            o2s = {}

            def outproj_part1():
                for oc in range(6):
                    ops = ps_proj.tile([128, S_CORE], F32, tag="proj")
                    for k in range(4):
                        nc.tensor.matmul(
                            ops[:],
                            WOT[:, k, oc, :],
                            AT[:, k * S_CORE:(k + 1) * S_CORE],
                            start=(k == 0), stop=(k == 3))
                    nc.scalar.copy(PO1[:, oc * S_CORE:(oc + 1) * S_CORE], ops[:])

            # ---- schedule ----
            ks = int(os.environ.get("KSTAGE", "0") or 0)
            vt_unit(0)
            vt_unit(1)
            proj_hp(0)
            vt_unit(2)
            vt_unit(3)
            proj_hp(1)
            vt_unit(4)
            if ks == 1:
                for hp in range(2, NHP):
                    proj_hp(hp)
            if ks == 0 or ks >= 2:
                WOT = sb.tile([128, NHP, 6, 128], BF16, tag="WOT")
                nc.sync.dma_start(
                    out=WOT[:],
                    in_=wot_d.ap().rearrange("p (k o m) -> p k o m", k=NHP, o=6))

                units = [{"hp": hp, "kc": kc}
                         for hp in range(NHP) for kc in range(NSC)]
                NU = len(units)
                stages = [stage_scores, stage_exp, stage_mask]
                if ks == 21:
                    stages = [stage_scores]
                elif ks == 22:
                    stages = [stage_scores, stage_exp]
                ND = len(stages)
                if ks == 25:
                    units = []
                    NU = 0
                    for hp in range(2, NHP):
                        proj_hp(hp)
                for step in range(NU + ND):
                    if step < NU and units[step]["kc"] == 0:
                        hp = units[step]["hp"]
                        if 2 <= hp + 2 < NHP:
                            proj_hp(hp + 2)
                    for k in range(ND - 1, -1, -1):
                        idx = step - k
                        if 0 <= idx < NU:
                            stages[k](units[idx])
                    # pv for the unit whose mask just completed
                    ipv = step - (ND - 1)
                    if 0 <= ipv < NU and units[ipv]["kc"] > 0:
                        if ks in (0, 3):
                            stage_pv(units[ipv], units[ipv - 1]["pm"])
                            if units[ipv]["kc"] == NSC - 1:
                                if ks == 0:
                                    finish_hp(units[ipv]["hp"])
                                    if units[ipv]["hp"] == 3:
                                        outproj_part1()
                                else:
                                    del o2s[units[ipv]["hp"]]

            if ks == 0:
                # ---- output projection tail: hp-4/5 chunks + combine ----
                for oc in range(6):
                    ops = ps_proj.tile([128, S_CORE], F32, tag="proj")
                    for k in (4, 5):
                        nc.tensor.matmul(
                            ops[:], WOT[:, k, oc, :],
                            AT[:, k * S_CORE:(k + 1) * S_CORE],
                            start=(k == 4), stop=(k == 5))
                    ot = outp.tile([128, S_CORE], BF16, tag="ot")
                    nc.vector.scalar_tensor_tensor(
                        out=ot[:], in0=ops[:], scalar=1.0,
                        in1=PO1[:, oc * S_CORE:(oc + 1) * S_CORE],
                        op0=mult, op1=addop)
                    nc.sync.dma_start(
                        out=out_d.ap()[:, oc * S_CORE:(oc + 1) * S_CORE],
                        in_=ot[:])
            else:
                for oc in range(6):
                    ot = outp.tile([128, S_CORE], BF16, tag="ot")
                    nc.vector.tensor_copy(
                        ot[:], QKa[:, oc * 512:(oc + 1) * 512])
                    nc.sync.dma_start(
                        out=out_d.ap()[:, oc * S_CORE:(oc + 1) * S_CORE],
                        in_=ot[:])

    nc.compile()
    return nc


def get_program(add_mask: bool, reps: int = 1, isv: int = 0):
    key = (add_mask, reps, isv)
    if key not in _BUILD_CACHE:
        _BUILD_CACHE[key] = _build(add_mask, isv, reps)
    return _BUILD_CACHE[key]


def _pow2(std):
    return float(2.0 ** np.round(np.log2(1.0 / (std + 1e-30))))


def _fp8_split(a):
    hi = a.astype(ml_dtypes.float8_e4m3fn)
    lo = (a - hi.astype(np.float32)).astype(ml_dtypes.float8_e4m3fn)
    return hi, lo


def _pack_pairs(a, width):
    """[768, width] -> [128, NPR, 2, width] with row = pr*256 + two*128 + p."""
    return np.ascontiguousarray(
        np.asarray(a).reshape(NPR, 2, 128, width).transpose(2, 0, 1, 3))


def prep_core_inputs(core, xs, pos, am, qkv_weight, out_weight, add_mask,
                     scales):
    sX, sQ, sK, sV = scales
    start = S_CORE * core - HALO
    idx = np.arange(start, start + SPAN)
    valid = (idx >= 0) & (idx < SEQ)

    Xs = np.zeros((HID, SPAN), np.float32)
    Xs[:, valid] = xs[:, idx[valid]]
    xhi, xlo = _fp8_split(Xs * sX)

    # rope tables: q part positions [HALO, HALO+512), k part [0, SPAN)
    pspan = np.zeros((SPAN,), np.float32)
    pspan[valid] = pos[idx[valid]]
    invf = (1.0 / (10000.0 ** (np.arange(0, DH, 2, dtype=np.float32)
                               / np.float32(DH)))).astype(np.float32)
    f = pspan[None, :] * invf[:, None]          # [32, SPAN]
    cos64 = np.tile(np.cos(f), (2, 1))          # [64, SPAN]
    sin64 = np.tile(np.sin(f), (2, 1))
    sgn = np.where(np.arange(DH) < DH // 2, -1.0, 1.0).astype(np.float32)
    sin64 = sin64 * sgn[:, None]
    cos128 = np.tile(cos64, (2, 1))             # [128, SPAN]
    sin128 = np.tile(sin64, (2, 1))
    iq = 1.0 / (sX * sQ)
    ik = 1.0 / (sX * sK)
    cosqk = np.concatenate(
        [cos128[:, HALO:HALO + S_CORE] * iq, cos128 * ik], 1)
    sinqk = np.concatenate(
        [sin128[:, HALO:HALO + S_CORE] * iq, sin128 * ik], 1)

    # band masks in [S(i), (h, j)] layout; in-band iff j-128 <= i <= j, plus
    # global key-existence at the sequence edges (kc=0 / kc=4 variants).
    i = np.arange(128)[:, None]
    j = np.arange(256)[None, :]
    band = ((j - 128 <= i) & (i <= j)).astype(np.float32)  # [128, 256]
    gkey0 = start + np.arange(128)
    gkey4 = start + 4 * 128 + np.arange(128)
    v0 = ((gkey0 >= 0) & (gkey0 < SEQ)).astype(np.float32)[:, None]
    v4 = ((gkey4 >= 0) & (gkey4 < SEQ)).astype(np.float32)[:, None]
    bandm = np.zeros((128, 3, 2, 256), np.float32)
    for h in range(2):
        bandm[:, 0, h] = band * v0
        bandm[:, 1, h] = band
        bandm[:, 2, h] = band * v4

    wq = qkv_weight[0:HID] * np.float32(DH ** -0.5)
    wk = qkv_weight[HID:2 * HID]
    wv = qkv_weight[2 * HID:3 * HID]

    def pack_w_hp(w, s):
        wt = np.ascontiguousarray(w.T) * s       # [768c, 768o]
        hi, lo = _fp8_split(wt)

        def hp_major(p):
            return np.ascontiguousarray(
                _pack_pairs(p, HID).reshape(128, NPR, 2, NHP, 128)
                .transpose(0, 3, 1, 2, 4)).reshape(128, NHP * NPR * 2 * 128)

        return hp_major(hi), hp_major(lo)

    wqh, wql = pack_w_hp(wq, sQ)
    wkh, wkl = pack_w_hp(wk, sK)
    # reorder wv columns so each vp half is [h0 of 3 hps | h1 of same]
    head_order = [hf * 6 + t for hf in range(2) for t in (0, 2, 4, 1, 3, 5)]
    col_perm = np.concatenate([np.arange(hg * 64, hg * 64 + 64)
                               for hg in head_order])
    vt_cols = np.ascontiguousarray(wv.T[:, col_perm]) * sV
    vhi, vlo = _fp8_split(vt_cols)
    wvh = _pack_pairs(vhi, HID).reshape(128, NPR * 2 * HID)
    wvl = _pack_pairs(vlo, HID).reshape(128, NPR * 2 * HID)

    # out_weight stationary: wot[p, k(hp), oc, j] = out_weight[oc*128+j, k*128+p]
    wot = np.ascontiguousarray(
        out_weight.reshape(6, 128, NHP, 128).transpose(3, 2, 0, 1)
    ).astype(ml_dtypes.bfloat16)

    xh_p = _pack_pairs(xhi, SPAN).reshape(128, NPR * 2 * SPAN)
    xl_p = _pack_pairs(xlo, SPAN).reshape(128, NPR * 2 * SPAN)
    in_map = {
        "xhl": np.ascontiguousarray(np.concatenate([xh_p, xl_p], 1)),
        "wqk": np.ascontiguousarray(np.concatenate([wqh, wql, wkh, wkl], 1)),
        "wvhl": np.ascontiguousarray(np.concatenate([wvh, wvl], 1)),
        "wot": np.ascontiguousarray(wot.reshape(128, NHP * 6 * 128)),
        "tabs": np.ascontiguousarray(np.concatenate(
            [cosqk.astype(ml_dtypes.bfloat16),
             sinqk.astype(ml_dtypes.bfloat16),
             bandm.reshape(128, 3 * 512).astype(ml_dtypes.bfloat16)], 1)),
    }
    if add_mask:
        mf = np.zeros((128, NSC, 2, 256), np.float32)
        for kc in range(NSC):
            qlo = max(0, kc * 128 - 128)
            qhi = min(S_CORE, kc * 128 + 128)
            off = 128 if kc == 0 else 0
            gq = S_CORE * core + np.arange(qlo, qhi)
            gk = start + kc * 128 + np.arange(128)
            kvalid = (gk >= 0) & (gk < SEQ)
            sub = np.zeros((128, qhi - qlo), np.float32)
            sub[kvalid, :] = am[np.ix_(gq, gk[kvalid])].T
            for h in range(2):
                mf[:, kc, h, off:off + qhi - qlo] = sub
        in_map["maskf"] = np.ascontiguousarray(mf.reshape(128, NSC * 512))
    return in_map


def prep_all_inputs(x, position_ids, attention_mask, qkv_weight, out_weight):
    xs = np.asarray(x, dtype=np.float32)[0, :, 0, :]
    pos = np.asarray(position_ids)[0].astype(np.float32)
    am = np.asarray(attention_mask, dtype=np.float32)[0, 0]
    qkv_w = np.asarray(qkv_weight, dtype=np.float32)
    out_w = np.asarray(out_weight, dtype=np.float32)
    add_mask = bool(np.any(am))
    sX = _pow2(xs.std())
    sQ = _pow2((qkv_w[0:HID] * np.float32(DH ** -0.5)).std())
    sK = _pow2(qkv_w[HID:2 * HID].std())
    sV = _pow2(qkv_w[2 * HID:3 * HID].std())
    scales = (sX, sQ, sK, sV)
    isv = int(np.round(np.log2(1.0 / (sX * sV))))
    in_maps = [
        prep_core_inputs(c, xs, pos, am, qkv_w, out_w, add_mask, scales)
        for c in range(N_CORES)
    ]
    return in_maps, add_mask, isv


def assemble_output(results):
    cols = []
    for c in range(N_CORES):
        o = np.asarray(results[c]["out"]).astype(np.float32)  # [128, 6*512]
        cols.append(
            o.reshape(128, 6, S_CORE).transpose(1, 0, 2).reshape(HID, S_CORE))
    full = np.concatenate(cols, axis=1)
    return np.ascontiguousarray(full.reshape(1, HID, 1, SEQ), dtype=np.float32)


def kernel(**inputs):
    in_maps, add_mask, isv = prep_all_inputs(
        inputs["x"], inputs["position_ids"], inputs["attention_mask"],
        inputs["qkv_weight"], inputs["out_weight"])
    nc = get_program(add_mask, isv=isv)
    res = run_bass_kernel_spmd(nc, in_maps, core_ids=list(range(N_CORES)))
    return assemble_output(res.results)
